# revision 15
# baseline (speedup 1.0000x reference)
"""Trainium2 Bass kernel for DAEEmbeddingModel inference.

Computes, given x_cat [2048, 32] int32 and DAE params:
    y  = sigmoid(sum_i W_enc[:, off_i + x_cat[:, i]] + b_enc)   # [2048, 128]
    z  = y @ W_dec.T + b_dec                                    # [2048, 64000]

Sharding over 8 NeuronCores: pure batch split (256 rows per core).
Each core:
  - encode: 33 indirect-DMA gathers per 128-row tile (one offset per
    partition, 512B row each) from the host-transposed table
    W_encT_plus [64001, 128] whose row 64000 is b_enc (an extra 33rd index
    column folds the encoder bias into the gather-sum); one DVE reduction
    sums the 33 rows; sigmoid on ACT; PE transposes y.
  - decode GEMM, K split 96+33: z = yT[0:96].T @ W1 + yTa.T @ W2 where
    W2's last row is b_dec and yTa's last row is ones, so the decoder bias
    is accumulated by the PE with no broadcast anywhere.
  - PSUM drains on DVE into staging tiles, ~1MB z stores.

Constraints honored: PE Matmult carries at most ONE semaphore wait on this
toolchain, so every matmul operand (incl. PSUM WAR) is funneled through the
DVE domain (weights DVE-copied after DMA, drains DVE-only).  Custom GPSIMD
ucode ops (dma_gather/partition_broadcast) are unavailable on this image;
only plain/indirect DMA, standard DVE/ACT/PE ops are used.

Host only does layout transforms (transposes, index offsets, shard
slicing/concat); all FLOPs and the gather happen on device.
"""

import os
from contextlib import ExitStack

import numpy as np

from concourse import bacc, bass, mybir
import concourse.tile as tile
from concourse.bass_utils import run_bass_kernel_spmd
from concourse.masks import make_identity

# ---- problem geometry (hardcoded per contest rules) ----
B = 2048          # batch
NCOL = 32         # categorical columns
CAT = 2000        # per-column vocab
V = NCOL * CAT    # 64000 total one-hot
H = 128           # hidden
N_CORES = 8

BL = B // N_CORES     # 256 local batch rows
NB = BL // 128        # 2 batch tiles of 128
AUG = NCOL + 1        # 33: 32 gathers + 1 bias-row gather
K1 = 96               # decode K split: 96 + (32 hidden + 1 bias row)
K2 = H - K1 + 1       # 33
CHW = 512             # decode chunk width (1 PSUM bank of f32)
NCH = V // CHW        # 125 chunks, exact
GROUP = 4             # chunks per staging/store group (2048 cols = 1MB)

F32 = mybir.dt.float32
I32 = mybir.dt.int32

_PROGRAM = None
LAST_RESULTS = None  # BassKernelResults of the most recent run (for test.py)


def build_program():
    """Build the SPMD Bass program (same code on all 8 cores)."""
    nc = bacc.Bacc("TRN2", target_bir_lowering=False, debug=False,
                   num_devices=N_CORES)

    gidx = nc.declare_dram_parameter("gidx", [BL, AUG], I32, isOutput=False)
    wenc = nc.declare_dram_parameter("wenc", [V + 1, H], F32, isOutput=False)
    wdec1 = nc.declare_dram_parameter("wdec1", [K1, V], F32, isOutput=False)
    wdec2 = nc.declare_dram_parameter("wdec2", [K2, V], F32, isOutput=False)
    y_out = nc.declare_dram_parameter("y_out", [BL, H], F32, isOutput=True)
    z_out = nc.declare_dram_parameter("z_out", [BL, V], F32, isOutput=True)

    with tile.TileContext(nc) as tc, ExitStack() as ctx:
        const_pool = ctx.enter_context(tc.tile_pool(name="const", bufs=1))
        emb_pool = ctx.enter_context(tc.tile_pool(name="emb", bufs=2))
        small = ctx.enter_context(tc.tile_pool(name="small", bufs=2))
        yt_pool = ctx.enter_context(tc.tile_pool(name="yt", bufs=1))
        w1_pool = ctx.enter_context(tc.tile_pool(name="w1", bufs=8))
        w2_pool = ctx.enter_context(tc.tile_pool(name="w2", bufs=8))
        w1f_pool = ctx.enter_context(tc.tile_pool(name="w1f", bufs=4))
        w2f_pool = ctx.enter_context(tc.tile_pool(name="w2f", bufs=4))
        st_pools = [
            ctx.enter_context(tc.tile_pool(name=f"st{b}", bufs=2))
            for b in range(NB)
        ]
        psum_pool = ctx.enter_context(
            tc.tile_pool(name="psum", bufs=8, space="PSUM"))

        identity_g = const_pool.tile([128, 128], F32)
        make_identity(nc, identity_g[:])
        identity = const_pool.tile([128, 128], F32)
        nc.vector.tensor_copy(identity[:], identity_g[:])

        idx_sb = const_pool.tile([128, NB * AUG], I32)
        nc.sync.dma_start(
            out=idx_sb[:].rearrange("p (b i) -> p b i", b=NB),
            in_=gidx[:].rearrange("(b p) i -> p b i", p=128))

        yT = yt_pool.tile([128, BL], F32)    # [hidden, local batch]
        yTa = yt_pool.tile([K2, BL], F32)    # [hidden 96..127 + ones, batch]
        nc.vector.memset(yTa[K2 - 1:K2, :], 1.0)

        # ---- encode: gathers -> sum(33) -> sigmoid -> transpose ----
        for bt in range(NB):
            emb = emb_pool.tile([128, AUG * H], F32)
            for i in range(AUG):
                nc.gpsimd.indirect_dma_start(
                    out=emb[:, i * H:(i + 1) * H],
                    out_offset=None,
                    in_=wenc[:],
                    in_offset=bass.IndirectOffsetOnAxis(
                        ap=idx_sb[:, bt * AUG + i: bt * AUG + i + 1], axis=0),
                )
            ypre = small.tile([128, H], F32, tag="ypre")
            nc.vector.reduce_sum(
                out=ypre[:],
                in_=emb[:].rearrange("p (i h) -> p h i", i=AUG),
                axis=mybir.AxisListType.X)
            ysig = small.tile([128, H], F32, tag="ysig")
            nc.scalar.activation(
                ysig[:], ypre[:], mybir.ActivationFunctionType.Sigmoid)
            nc.sync.dma_start(
                out=y_out[bt * 128:(bt + 1) * 128, :], in_=ysig[:])
            ysig2 = small.tile([128, H], F32, tag="ysig2")
            nc.vector.tensor_copy(ysig2[:], ysig[:])
            ptr = psum_pool.tile([128, 128], F32, tag="ps", name="ptr")
            nc.tensor.transpose(ptr[:], ysig2[:], identity[:])
            nc.vector.tensor_copy(yT[:, bt * 128:(bt + 1) * 128], ptr[:])
            # second small transpose: hidden rows 96..127 at partition base 0
            ptr2 = psum_pool.tile([K2 - 1, 128], F32, tag="ps", name="ptr2")
            nc.tensor.transpose(ptr2[:], ysig2[:, K1:H], identity[:])
            nc.vector.tensor_copy(
                yTa[0:K2 - 1, bt * 128:(bt + 1) * 128], ptr2[:])

        # ---- decode: z[bt, ch] = yT1.T @ W1 + yTa.T @ [W2; b_dec] ----
        stage_tiles = [None] * NB
        for ch in range(NCH):
            col0 = ch * CHW
            g = ch % GROUP
            gcol0 = (ch // GROUP) * GROUP * CHW
            gw = min(GROUP * CHW, V - gcol0)

            w1_t = w1_pool.tile([K1, CHW], F32, tag="w1")
            nc.scalar.dma_start(out=w1_t[:], in_=wdec1[:, col0:col0 + CHW])
            w1_f = w1f_pool.tile([K1, CHW], F32, tag="w1f")
            nc.vector.tensor_copy(w1_f[:], w1_t[:])
            w2_t = w2_pool.tile([K2, CHW], F32, tag="w2")
            nc.sync.dma_start(out=w2_t[:], in_=wdec2[:, col0:col0 + CHW])
            w2_f = w2f_pool.tile([K2, CHW], F32, tag="w2f")
            nc.vector.tensor_copy(w2_f[:], w2_t[:])
            for bt in range(NB):
                if g == 0:
                    stage_tiles[bt] = st_pools[bt].tile(
                        [128, GROUP * CHW], F32, tag="st", name=f"stage{bt}")
                ps = psum_pool.tile([128, CHW], F32, tag="ps")
                nc.tensor.matmul(ps[:],
                                 lhsT=yT[0:K1, bt * 128:(bt + 1) * 128],
                                 rhs=w1_f[:],
                                 start=True, stop=False)
                nc.tensor.matmul(ps[:],
                                 lhsT=yTa[:, bt * 128:(bt + 1) * 128],
                                 rhs=w2_f[:],
                                 start=False, stop=True)
                nc.vector.tensor_copy(
                    stage_tiles[bt][:, g * CHW:(g + 1) * CHW], ps[:])
                if g == GROUP - 1 or ch == NCH - 1:
                    nc.sync.dma_start(
                        out=z_out[bt * 128:(bt + 1) * 128, gcol0:gcol0 + gw],
                        in_=stage_tiles[bt][:, :gw])
    nc.compile()
    return nc


def get_program():
    global _PROGRAM
    if _PROGRAM is None:
        _PROGRAM = build_program()
    return _PROGRAM


def make_in_maps(x_cat, W_enc, b_enc, W_dec, b_dec):
    """Host-side layout prep + sharding. Returns list of 8 per-core dicts."""
    x_cat = np.ascontiguousarray(np.asarray(x_cat, dtype=np.int32))
    W_enc = np.asarray(W_enc, dtype=np.float32)
    b_enc = np.asarray(b_enc, dtype=np.float32)
    W_dec = np.asarray(W_dec, dtype=np.float32)
    b_dec = np.asarray(b_dec, dtype=np.float32)

    offs = (np.arange(NCOL, dtype=np.int32) * CAT)[None, :]
    gidx = x_cat + offs                                   # [B, 32] in [0, V)
    gidx_aug = np.concatenate(
        [gidx, np.full((B, 1), V, dtype=np.int32)], axis=1)  # bias row idx

    wenc_plus = np.ascontiguousarray(
        np.concatenate([W_enc.T, b_enc[None, :]], axis=0))   # [V+1, H]
    wdecT = W_dec.T                                          # [H, V] view
    wdec1 = np.ascontiguousarray(wdecT[:K1])                 # [96, V]
    wdec2 = np.ascontiguousarray(
        np.concatenate([wdecT[K1:], b_dec[None, :]], axis=0))  # [33, V]

    in_maps = []
    for c in range(N_CORES):
        in_maps.append({
            "gidx": np.ascontiguousarray(gidx_aug[c * BL:(c + 1) * BL]),
            "wenc": wenc_plus,
            "wdec1": wdec1,
            "wdec2": wdec2,
        })
    return in_maps


def kernel(x_cat, W_enc, b_enc, W_dec, b_dec):
    global LAST_RESULTS
    nc = get_program()
    in_maps = make_in_maps(x_cat, W_enc, b_enc, W_dec, b_dec)
    trace = bool(int(os.environ.get("KERNEL_TRACE", "0")))
    res = run_bass_kernel_spmd(nc, in_maps, list(range(N_CORES)),
                               trace=trace)
    LAST_RESULTS = res

    y = np.empty((B, H), dtype=np.float32)
    z = np.empty((B, V), dtype=np.float32)
    for c in range(N_CORES):
        out = res.results[c]
        y[c * BL:(c + 1) * BL] = out["y_out"]
        z[c * BL:(c + 1) * BL] = out["z_out"]
    return y, z


# revision 18
# speedup vs baseline: 2.6223x; 2.6223x over previous
"""Trainium2 Bass kernel for DAEEmbeddingModel inference.

Computes, given x_cat [2048, 32] int32 and DAE params:
    y  = sigmoid(sum_i W_enc[:, off_i + x_cat[:, i]] + b_enc)   # [2048, 128]
    z  = y @ W_dec.T + b_dec                                    # [2048, 64000]

Sharding over 8 NeuronCores: pure batch split (256 rows per core).
Each core:
  - encode: 33 indirect-DMA gathers per 128-row tile (one offset per
    partition, 512B row each) from the host-transposed table
    W_encT_plus [64001, 128] whose row 64000 is b_enc (an extra 33rd index
    column folds the encoder bias into the gather-sum); one DVE reduction
    sums the 33 rows; sigmoid on ACT (y stays fp32); PE transposes y.
  - decode GEMM in bf16 (fp32 PSUM accumulation; fp32 PE runs ~3.5x slower
    and becomes the bottleneck), K split 96+33:
    z = yT[0:96].T @ W1 + yTa.T @ W2 where W2's last row is b_dec and yTa's
    last row is ones, so the decoder bias is accumulated on the PE with no
    broadcast op anywhere.
  - PSUM drains into staging tiles and weight-funnel copies alternate
    between DVE and ACT by chunk parity ("lanes"), ~1MB z stores.

Constraints honored: PE Matmult carries at most ONE semaphore wait on this
toolchain, so every recurring matmul dependency (weights funnel, PSUM WAR)
lives in a single engine domain per lane; PSUM slot rotation is
lane-consistent (8 slots, 2 per chunk -> reuse distance 4 chunks keeps
parity).  Custom GPSIMD ucode ops (dma_gather/partition_broadcast) are
unavailable on this image; only plain/indirect DMA and standard ops used.

Host only does layout transforms (transposes, index offsets, bf16 casts,
shard slicing/concat); all FLOPs and the gather happen on device.

Env: KERNEL_F32=1 switches the decode GEMM back to fp32 (slower, exact).
"""

import os
from contextlib import ExitStack

import ml_dtypes
import numpy as np

from concourse import bacc, bass, mybir
import concourse.tile as tile
from concourse.bass_utils import run_bass_kernel_spmd
from concourse.masks import make_identity

# ---- problem geometry (hardcoded per contest rules) ----
B = 2048          # batch
NCOL = 32         # categorical columns
CAT = 2000        # per-column vocab
V = NCOL * CAT    # 64000 total one-hot
H = 128           # hidden
N_CORES = 8

BL = B // N_CORES     # 256 local batch rows
NB = BL // 128        # 2 batch tiles of 128
AUG = NCOL + 1        # 33: 32 gathers + 1 bias-row gather
K1 = 96               # decode K split: 96 + (32 hidden + 1 bias row)
K2 = H - K1 + 1       # 33
CHW = 512             # decode chunk width (1 PSUM bank of f32)
NCH = V // CHW        # 125 chunks, exact
GROUP = 4             # chunks per staging/store group (2048 cols = 1MB)
GW = GROUP * CHW      # 2048

F32 = mybir.dt.float32
I32 = mybir.dt.int32

USE_F32 = bool(int(os.environ.get("KERNEL_F32", "0")))
WDT = F32 if USE_F32 else mybir.dt.bfloat16
NP_WDT = np.float32 if USE_F32 else ml_dtypes.bfloat16

_PROGRAM = None
LAST_RESULTS = None  # BassKernelResults of the most recent run (for test.py)


def build_program():
    """Build the SPMD Bass program (same code on all 8 cores)."""
    nc = bacc.Bacc("TRN2", target_bir_lowering=False, debug=False,
                   num_devices=N_CORES)

    gidx = nc.declare_dram_parameter("gidx", [BL, AUG], I32, isOutput=False)
    wenc = nc.declare_dram_parameter("wenc", [V + 1, H], F32, isOutput=False)
    wdec1 = nc.declare_dram_parameter("wdec1", [K1, V], WDT, isOutput=False)
    wdec2 = nc.declare_dram_parameter("wdec2", [K2, V], WDT, isOutput=False)
    y_out = nc.declare_dram_parameter("y_out", [BL, H], F32, isOutput=True)
    z_out = nc.declare_dram_parameter("z_out", [BL, V], F32, isOutput=True)

    with tile.TileContext(nc) as tc, ExitStack() as ctx:
        const_pool = ctx.enter_context(tc.tile_pool(name="const", bufs=1))
        emb_pool = ctx.enter_context(tc.tile_pool(name="emb", bufs=2))
        small = ctx.enter_context(tc.tile_pool(name="small", bufs=2))
        yt_pool = ctx.enter_context(tc.tile_pool(name="yt", bufs=1))
        w1_pool = ctx.enter_context(tc.tile_pool(name="w1", bufs=4))
        w2_pool = ctx.enter_context(tc.tile_pool(name="w2", bufs=4))
        w1f_pool = ctx.enter_context(tc.tile_pool(name="w1f", bufs=4))
        w2f_pool = ctx.enter_context(tc.tile_pool(name="w2f", bufs=4))
        st_pools = [
            ctx.enter_context(tc.tile_pool(name=f"st{b}", bufs=2))
            for b in range(NB)
        ]
        psum_pool = ctx.enter_context(
            tc.tile_pool(name="psum", bufs=8, space="PSUM"))

        identity_g = const_pool.tile([128, 128], WDT)
        make_identity(nc, identity_g[:])
        identity = const_pool.tile([128, 128], WDT)
        nc.vector.tensor_copy(identity[:], identity_g[:])

        idx_sb = const_pool.tile([128, NB * AUG], I32)
        nc.sync.dma_start(
            out=idx_sb[:].rearrange("p (b i) -> p b i", b=NB),
            in_=gidx[:].rearrange("(b p) i -> p b i", p=128))

        yT = yt_pool.tile([128, BL], WDT)    # [hidden, local batch]
        yTa = yt_pool.tile([K2, BL], WDT)    # [hidden 96..127 + ones, batch]
        nc.vector.memset(yTa[K2 - 1:K2, :], 1.0)

        # ---- encode: gathers -> sum(33) -> sigmoid -> transpose ----
        for bt in range(NB):
            emb = emb_pool.tile([128, AUG * H], F32)
            for i in range(AUG):
                nc.gpsimd.indirect_dma_start(
                    out=emb[:, i * H:(i + 1) * H],
                    out_offset=None,
                    in_=wenc[:],
                    in_offset=bass.IndirectOffsetOnAxis(
                        ap=idx_sb[:, bt * AUG + i: bt * AUG + i + 1], axis=0),
                )
            ypre = small.tile([128, H], F32, tag="ypre")
            nc.vector.reduce_sum(
                out=ypre[:],
                in_=emb[:].rearrange("p (i h) -> p h i", i=AUG),
                axis=mybir.AxisListType.X)
            ysig = small.tile([128, H], F32, tag="ysig")
            nc.scalar.activation(
                ysig[:], ypre[:], mybir.ActivationFunctionType.Sigmoid)
            nc.sync.dma_start(
                out=y_out[bt * 128:(bt + 1) * 128, :], in_=ysig[:])
            ysig2 = small.tile([128, H], WDT, tag="ysig2")
            nc.vector.tensor_copy(ysig2[:], ysig[:])
            ptr = psum_pool.tile([128, 128], WDT, tag="ps", name="ptr")
            nc.tensor.transpose(ptr[:], ysig2[:], identity[:])
            nc.vector.tensor_copy(yT[:, bt * 128:(bt + 1) * 128], ptr[:])
            # second small transpose: hidden rows 96..127 at partition base 0
            ptr2 = psum_pool.tile([K2 - 1, 128], WDT, tag="ps", name="ptr2")
            nc.tensor.transpose(ptr2[:], ysig2[:, K1:H], identity[:])
            nc.vector.tensor_copy(
                yTa[0:K2 - 1, bt * 128:(bt + 1) * 128], ptr2[:])

        # ---- decode: z[bt, ch] = yT1.T @ W1 + yTa.T @ [W2; b_dec] ----
        stage_tiles = [None] * NB
        w1g = w2g = None
        for ch in range(NCH):
            col0 = ch * CHW
            g = ch % GROUP
            gcol0 = (ch // GROUP) * GW
            gw = min(GW, V - gcol0)
            # DVE lane on even chunks, ACT lane on odd chunks
            eng = nc.vector if ch % 2 == 0 else nc.scalar
            copy = (nc.vector.tensor_copy if ch % 2 == 0 else nc.scalar.copy)

            if g == 0:
                w1g = w1_pool.tile([K1, GW], WDT, tag="w1", name="w1g")
                nc.scalar.dma_start(out=w1g[:, :gw],
                                    in_=wdec1[:, gcol0:gcol0 + gw])
                w2g = w2_pool.tile([K2, GW], WDT, tag="w2", name="w2g")
                nc.scalar.dma_start(out=w2g[:, :gw],
                                    in_=wdec2[:, gcol0:gcol0 + gw])
            w1_f = w1f_pool.tile([K1, CHW], WDT, tag="w1f")
            copy(w1_f[:], w1g[:, g * CHW:(g + 1) * CHW])
            w2_f = w2f_pool.tile([K2, CHW], WDT, tag="w2f")
            copy(w2_f[:], w2g[:, g * CHW:(g + 1) * CHW])
            for bt in range(NB):
                if g == 0:
                    stage_tiles[bt] = st_pools[bt].tile(
                        [128, GW], F32, tag="st", name=f"stage{bt}")
                ps = psum_pool.tile([128, CHW], F32, tag="ps")
                nc.tensor.matmul(ps[:],
                                 lhsT=yT[0:K1, bt * 128:(bt + 1) * 128],
                                 rhs=w1_f[:],
                                 start=True, stop=False)
                nc.tensor.matmul(ps[:],
                                 lhsT=yTa[:, bt * 128:(bt + 1) * 128],
                                 rhs=w2_f[:],
                                 start=False, stop=True)
                copy(stage_tiles[bt][:, g * CHW:(g + 1) * CHW], ps[:])
                if g == GROUP - 1 or ch == NCH - 1:
                    nc.sync.dma_start(
                        out=z_out[bt * 128:(bt + 1) * 128, gcol0:gcol0 + gw],
                        in_=stage_tiles[bt][:, :gw])
    nc.compile()
    return nc


def get_program():
    global _PROGRAM
    if _PROGRAM is None:
        _PROGRAM = build_program()
    return _PROGRAM


def make_in_maps(x_cat, W_enc, b_enc, W_dec, b_dec):
    """Host-side layout prep + sharding. Returns list of 8 per-core dicts."""
    x_cat = np.ascontiguousarray(np.asarray(x_cat, dtype=np.int32))
    W_enc = np.asarray(W_enc, dtype=np.float32)
    b_enc = np.asarray(b_enc, dtype=np.float32)
    W_dec = np.asarray(W_dec, dtype=np.float32)
    b_dec = np.asarray(b_dec, dtype=np.float32)

    offs = (np.arange(NCOL, dtype=np.int32) * CAT)[None, :]
    gidx = x_cat + offs                                   # [B, 32] in [0, V)
    gidx_aug = np.concatenate(
        [gidx, np.full((B, 1), V, dtype=np.int32)], axis=1)  # bias row idx

    wenc_plus = np.ascontiguousarray(
        np.concatenate([W_enc.T, b_enc[None, :]], axis=0))   # [V+1, H]
    wdecT = W_dec.T                                          # [H, V] view
    wdec1 = np.ascontiguousarray(wdecT[:K1]).astype(NP_WDT)  # [96, V]
    wdec2 = np.concatenate([wdecT[K1:], b_dec[None, :]],
                           axis=0).astype(NP_WDT)            # [33, V]

    in_maps = []
    for c in range(N_CORES):
        in_maps.append({
            "gidx": np.ascontiguousarray(gidx_aug[c * BL:(c + 1) * BL]),
            "wenc": wenc_plus,
            "wdec1": wdec1,
            "wdec2": wdec2,
        })
    return in_maps


def kernel(x_cat, W_enc, b_enc, W_dec, b_dec):
    global LAST_RESULTS
    nc = get_program()
    in_maps = make_in_maps(x_cat, W_enc, b_enc, W_dec, b_dec)
    trace = bool(int(os.environ.get("KERNEL_TRACE", "0")))
    res = run_bass_kernel_spmd(nc, in_maps, list(range(N_CORES)),
                               trace=trace)
    LAST_RESULTS = res

    y = np.empty((B, H), dtype=np.float32)
    z = np.empty((B, V), dtype=np.float32)
    for c in range(N_CORES):
        out = res.results[c]
        y[c * BL:(c + 1) * BL] = out["y_out"]
        z[c * BL:(c + 1) * BL] = out["z_out"]
    return y, z


# revision 21
# speedup vs baseline: 2.8069x; 1.0704x over previous
"""Trainium2 Bass kernel for DAEEmbeddingModel inference.

Computes, given x_cat [2048, 32] int32 and DAE params:
    y  = sigmoid(sum_i W_enc[:, off_i + x_cat[:, i]] + b_enc)   # [2048, 128]
    z  = y @ W_dec.T + b_dec                                    # [2048, 64000]

Sharding over 8 NeuronCores: pure batch split (256 rows per core).
Each core:
  - encode: 33 indirect-DMA gathers per 128-row tile (one offset per
    partition, 512B row each) from the host-transposed table
    W_encT_plus [64001, 128] whose row 64000 is b_enc (an extra 33rd index
    column folds the encoder bias into the gather-sum); one DVE reduction
    sums the 33 rows; sigmoid on ACT (y stays fp32); PE transposes y.
  - decode GEMM in bf16 (fp32 PSUM accumulation; fp32 PE runs ~3.5x slower
    and becomes the bottleneck), K split 96+33:
    z = yT[0:96].T @ W1 + yTa.T @ W2 where W2's last row is b_dec and yTa's
    last row is ones, so the decoder bias is accumulated on the PE with no
    broadcast op anywhere.
  - PSUM drains into staging tiles and weight-funnel copies alternate
    between DVE and ACT by chunk parity ("lanes"), ~1MB z stores.

Constraints honored: PE Matmult carries at most ONE semaphore wait on this
toolchain, so every recurring matmul dependency (weights funnel, PSUM WAR)
lives in a single engine domain per lane; PSUM slot rotation is
lane-consistent (8 slots, 2 per chunk -> reuse distance 4 chunks keeps
parity).  Custom GPSIMD ucode ops (dma_gather/partition_broadcast) are
unavailable on this image; only plain/indirect DMA and standard ops used.

Host only does layout transforms (transposes, index offsets, bf16 casts,
shard slicing/concat); all FLOPs and the gather happen on device.

Env: KERNEL_F32=1 switches the decode GEMM back to fp32 (slower, exact).
"""

import os
from contextlib import ExitStack

import ml_dtypes
import numpy as np

from concourse import bacc, bass, mybir
import concourse.tile as tile
from concourse.bass_utils import run_bass_kernel_spmd
from concourse.masks import make_identity

# ---- problem geometry (hardcoded per contest rules) ----
B = 2048          # batch
NCOL = 32         # categorical columns
CAT = 2000        # per-column vocab
V = NCOL * CAT    # 64000 total one-hot
H = 128           # hidden
N_CORES = 8

BL = B // N_CORES     # 256 local batch rows
NB = BL // 128        # 2 batch tiles of 128
AUG = NCOL + 1        # 33: 32 gathers + 1 bias-row gather
K1 = 96               # decode K split: 96 + (32 hidden + 1 bias row)
K2 = H - K1 + 1       # 33
CHW = 512             # decode chunk width (1 PSUM bank of f32)
NCH = V // CHW        # 125 chunks, exact
GROUP = 4             # chunks per staging/store group (2048 cols = 1MB)
GW = GROUP * CHW      # 2048

F32 = mybir.dt.float32
I32 = mybir.dt.int32

USE_F32 = bool(int(os.environ.get("KERNEL_F32", "0")))
WDT = F32 if USE_F32 else mybir.dt.bfloat16
NP_WDT = np.float32 if USE_F32 else ml_dtypes.bfloat16

_PROGRAM = None
LAST_RESULTS = None  # BassKernelResults of the most recent run (for test.py)


def build_program():
    """Build the SPMD Bass program (same code on all 8 cores)."""
    nc = bacc.Bacc("TRN2", target_bir_lowering=False, debug=False,
                   num_devices=N_CORES)

    gidx = nc.declare_dram_parameter("gidx", [BL, AUG], I32, isOutput=False)
    wenc = nc.declare_dram_parameter("wenc", [V + 1, H], F32, isOutput=False)
    wdec1 = nc.declare_dram_parameter("wdec1", [K1, V], WDT, isOutput=False)
    wdec2 = nc.declare_dram_parameter("wdec2", [K2, V], WDT, isOutput=False)
    y_out = nc.declare_dram_parameter("y_out", [BL, H], F32, isOutput=True)
    z_out = nc.declare_dram_parameter("z_out", [BL, V], F32, isOutput=True)

    with tile.TileContext(nc) as tc, ExitStack() as ctx:
        const_pool = ctx.enter_context(tc.tile_pool(name="const", bufs=1))
        emb_pool = ctx.enter_context(tc.tile_pool(name="emb", bufs=2))
        small = ctx.enter_context(tc.tile_pool(name="small", bufs=2))
        yt_pool = ctx.enter_context(tc.tile_pool(name="yt", bufs=1))
        w1_pool = ctx.enter_context(tc.tile_pool(name="w1", bufs=5))
        w2_pool = ctx.enter_context(tc.tile_pool(name="w2", bufs=5))
        w1f_pool = ctx.enter_context(tc.tile_pool(name="w1f", bufs=6))
        w2f_pool = ctx.enter_context(tc.tile_pool(name="w2f", bufs=6))
        st_pools = [
            ctx.enter_context(tc.tile_pool(name=f"st{b}", bufs=2))
            for b in range(NB)
        ]
        psum_pool = ctx.enter_context(
            tc.tile_pool(name="psum", bufs=8, space="PSUM"))

        identity_g = const_pool.tile([128, 128], WDT)
        make_identity(nc, identity_g[:])
        identity = const_pool.tile([128, 128], WDT)
        nc.vector.tensor_copy(identity[:], identity_g[:])

        idx_sb = const_pool.tile([128, NB * AUG], I32)
        nc.sync.dma_start(
            out=idx_sb[:].rearrange("p (b i) -> p b i", b=NB),
            in_=gidx[:].rearrange("(b p) i -> p b i", p=128))

        yT = yt_pool.tile([128, BL], WDT)    # [hidden, local batch]
        yTa = yt_pool.tile([K2, BL], WDT)    # [hidden 96..127 + ones, batch]
        nc.vector.memset(yTa[K2 - 1:K2, :], 1.0)

        # prefetch decoder weight groups so they stream during the encode
        NGRP = (NCH + GROUP - 1) // GROUP
        w_groups = {}

        def load_group(gi):
            gcol = gi * GW
            gwid = min(GW, V - gcol)
            w1g = w1_pool.tile([K1, GW], WDT, tag="w1", name="w1g")
            nc.scalar.dma_start(out=w1g[:, :gwid],
                                in_=wdec1[:, gcol:gcol + gwid])
            w2g = w2_pool.tile([K2, GW], WDT, tag="w2", name="w2g")
            nc.scalar.dma_start(out=w2g[:, :gwid],
                                in_=wdec2[:, gcol:gcol + gwid])
            w_groups[gi] = (w1g, w2g)

        PREFETCH = 4
        for gi in range(min(PREFETCH, NGRP)):
            load_group(gi)

        # ---- encode: gathers -> sum(33) -> sigmoid -> transpose ----
        for bt in range(NB):
            emb = emb_pool.tile([128, AUG * H], F32)
            for i in range(AUG):
                nc.gpsimd.indirect_dma_start(
                    out=emb[:, i * H:(i + 1) * H],
                    out_offset=None,
                    in_=wenc[:],
                    in_offset=bass.IndirectOffsetOnAxis(
                        ap=idx_sb[:, bt * AUG + i: bt * AUG + i + 1], axis=0),
                )
            ypre = small.tile([128, H], F32, tag="ypre")
            nc.vector.reduce_sum(
                out=ypre[:],
                in_=emb[:].rearrange("p (i h) -> p h i", i=AUG),
                axis=mybir.AxisListType.X)
            ysig = small.tile([128, H], F32, tag="ysig")
            nc.scalar.activation(
                ysig[:], ypre[:], mybir.ActivationFunctionType.Sigmoid)
            nc.sync.dma_start(
                out=y_out[bt * 128:(bt + 1) * 128, :], in_=ysig[:])
            ysig2 = small.tile([128, H], WDT, tag="ysig2")
            nc.vector.tensor_copy(ysig2[:], ysig[:])
            ptr = psum_pool.tile([128, 128], WDT, tag="ps", name="ptr")
            nc.tensor.transpose(ptr[:], ysig2[:], identity[:])
            nc.vector.tensor_copy(yT[:, bt * 128:(bt + 1) * 128], ptr[:])
            # second small transpose: hidden rows 96..127 at partition base 0
            ptr2 = psum_pool.tile([K2 - 1, 128], WDT, tag="ps", name="ptr2")
            nc.tensor.transpose(ptr2[:], ysig2[:, K1:H], identity[:])
            nc.vector.tensor_copy(
                yTa[0:K2 - 1, bt * 128:(bt + 1) * 128], ptr2[:])

        # ---- decode: z[bt, ch] = yT1.T @ W1 + yTa.T @ [W2; b_dec] ----
        stage_tiles = [None] * NB
        for ch in range(NCH):
            col0 = ch * CHW
            g = ch % GROUP
            gi = ch // GROUP
            gcol0 = gi * GW
            gw = min(GW, V - gcol0)
            # DVE lane on even chunks, ACT lane on odd chunks
            copy = (nc.vector.tensor_copy if ch % 2 == 0 else nc.scalar.copy)

            if g == 0:
                if gi + PREFETCH < NGRP:
                    load_group(gi + PREFETCH)
                if gi - 1 in w_groups:
                    del w_groups[gi - 1]
            w1g, w2g = w_groups[gi]
            w1_f = w1f_pool.tile([K1, CHW], WDT, tag="w1f")
            copy(w1_f[:], w1g[:, g * CHW:(g + 1) * CHW])
            w2_f = w2f_pool.tile([K2, CHW], WDT, tag="w2f")
            copy(w2_f[:], w2g[:, g * CHW:(g + 1) * CHW])
            for bt in range(NB):
                if g == 0:
                    stage_tiles[bt] = st_pools[bt].tile(
                        [128, GW], F32, tag="st", name=f"stage{bt}")
                ps = psum_pool.tile([128, CHW], F32, tag="ps")
                nc.tensor.matmul(ps[:],
                                 lhsT=yT[0:K1, bt * 128:(bt + 1) * 128],
                                 rhs=w1_f[:],
                                 start=True, stop=False)
                nc.tensor.matmul(ps[:],
                                 lhsT=yTa[:, bt * 128:(bt + 1) * 128],
                                 rhs=w2_f[:],
                                 start=False, stop=True)
                copy(stage_tiles[bt][:, g * CHW:(g + 1) * CHW], ps[:])
                if g == GROUP - 1 or ch == NCH - 1:
                    nc.sync.dma_start(
                        out=z_out[bt * 128:(bt + 1) * 128, gcol0:gcol0 + gw],
                        in_=stage_tiles[bt][:, :gw])
    nc.compile()
    return nc


def get_program():
    global _PROGRAM
    if _PROGRAM is None:
        _PROGRAM = build_program()
    return _PROGRAM


def make_in_maps(x_cat, W_enc, b_enc, W_dec, b_dec):
    """Host-side layout prep + sharding. Returns list of 8 per-core dicts."""
    x_cat = np.ascontiguousarray(np.asarray(x_cat, dtype=np.int32))
    W_enc = np.asarray(W_enc, dtype=np.float32)
    b_enc = np.asarray(b_enc, dtype=np.float32)
    W_dec = np.asarray(W_dec, dtype=np.float32)
    b_dec = np.asarray(b_dec, dtype=np.float32)

    offs = (np.arange(NCOL, dtype=np.int32) * CAT)[None, :]
    gidx = x_cat + offs                                   # [B, 32] in [0, V)
    gidx_aug = np.concatenate(
        [gidx, np.full((B, 1), V, dtype=np.int32)], axis=1)  # bias row idx

    wenc_plus = np.ascontiguousarray(
        np.concatenate([W_enc.T, b_enc[None, :]], axis=0))   # [V+1, H]
    wdecT = W_dec.T                                          # [H, V] view
    wdec1 = np.ascontiguousarray(wdecT[:K1]).astype(NP_WDT)  # [96, V]
    wdec2 = np.concatenate([wdecT[K1:], b_dec[None, :]],
                           axis=0).astype(NP_WDT)            # [33, V]

    in_maps = []
    for c in range(N_CORES):
        in_maps.append({
            "gidx": np.ascontiguousarray(gidx_aug[c * BL:(c + 1) * BL]),
            "wenc": wenc_plus,
            "wdec1": wdec1,
            "wdec2": wdec2,
        })
    return in_maps


def kernel(x_cat, W_enc, b_enc, W_dec, b_dec):
    global LAST_RESULTS
    nc = get_program()
    in_maps = make_in_maps(x_cat, W_enc, b_enc, W_dec, b_dec)
    trace = bool(int(os.environ.get("KERNEL_TRACE", "0")))
    res = run_bass_kernel_spmd(nc, in_maps, list(range(N_CORES)),
                               trace=trace)
    LAST_RESULTS = res

    y = np.empty((B, H), dtype=np.float32)
    z = np.empty((B, V), dtype=np.float32)
    for c in range(N_CORES):
        out = res.results[c]
        y[c * BL:(c + 1) * BL] = out["y_out"]
        z[c * BL:(c + 1) * BL] = out["z_out"]
    return y, z
